# revision 15
# baseline (speedup 1.0000x reference)
"""Multi-head causal self-attention (B=4, T=2048, C=1024, H=16, D=64) on 8
Trainium2 NeuronCores — v2.

Sharding: hybrid (batch x head-half). Core c handles batch c//2 and heads
[8*(c%2), 8*(c%2)+8). Each core computes q/k/v projections for its 8 heads,
causal attention, and its half of the output projection rows (512 of 1024).
The host sums each batch's two partial projections and adds the bias.

Vs v1 (tensor-parallel over heads only):
- bf16 activations/weights on chip (f32 PSUM accumulate): halves DMA + SBUF.
- Per-core DMA drops 64MB -> ~16MB (x slice 4MB bf16, out partial 8MB f32).
- Score matmuls for a head-pair sit at partitions 0-63 / 64-127, so the PE
  runs them concurrently in disjoint row-groups (auto tile_position).
- Diagonal blocks narrowed: only columns [128m:512] are computed/exp'd, and
  the causal mask collapses to a single [128,128] triangular mask applied to
  the 128-column diagonal sub-block.
- Softmax normalization broadcast (1/denom over 64 partitions) done with a
  tiny PE matmul (ones[1,64].T @ rec[1,512]) instead of a DRAM bounce.

A post-pass splits excess semaphore waits (walrus accepts one sync wait per
instruction for several instruction structs); excess waits move onto injected
NoOps on the same engine queue.
"""
import functools

import numpy as np

import concourse.bass as bass
import concourse.mybir as mybir
import concourse.tile as tile
from concourse.masks import make_identity

F32 = mybir.dt.float32
F32R = mybir.dt.float32r
BF16 = mybir.dt.bfloat16

C, H, D = 1024, 16, 64
NCORES = 8
HPC = 8                    # heads per core
NPAIR = HPC // 2           # head-pairs per core = 4
CK = C // 128              # contraction chunks = 8
ActF = mybir.ActivationFunctionType


def split_excess_waits(nc):
    """Move all-but-one sync wait of every instruction onto injected NoOps."""
    n_split = 0
    for f in nc.m.functions:
        for blk in f.blocks:
            out, changed = [], False
            for inst in blk.instructions:
                si = inst.sync_info
                if si is not None and len(si.on_wait) > 1:
                    for w_ in si.on_wait[:-1]:
                        nop = mybir.InstNoOp(name=f"I-wsplit-{n_split}")
                        n_split += 1
                        nop.engine = inst.engine
                        nop.sync_info = mybir.SyncInfo(on_wait=[w_], on_update=[])
                        out.append(nop)
                    inst.sync_info = mybir.SyncInfo(
                        on_wait=si.on_wait[-1:], on_update=si.on_update)
                    changed = True
                out.append(inst)
            if changed:
                blk.instructions = out
    return n_split


def build_nc(B, T):
    """One SPMD program; all 8 cores run it on different data slices."""
    NIB = T // 512           # query row-blocks = 4
    NJT = T // 128           # key blocks = 16
    nc = bass.Bass()

    xt_d = nc.dram_tensor("xt", [CK, 128, T], BF16, kind="ExternalInput")
    wqkv_d = nc.dram_tensor("wqkv", [CK, 128, 3, HPC * D], BF16,
                            kind="ExternalInput")
    wp_d = nc.dram_tensor("wp", [128, NPAIR, C], BF16, kind="ExternalInput")
    out_d = nc.dram_tensor("out", [T, C], BF16, kind="ExternalOutput")

    with tile.TileContext(nc) as tc:
        with (
            tc.tile_pool(name="consts", bufs=1) as consts,
            tc.tile_pool(name="xtp", bufs=3) as xtp,
            tc.tile_pool(name="qkv", bufs=1) as qkv,
            tc.tile_pool(name="vnp", bufs=1) as vnp,
            tc.tile_pool(name="pp", bufs=8) as ppool,
            tc.tile_pool(name="nrm", bufs=6) as nrm,
            tc.tile_pool(name="yp", bufs=3) as ypool,
            tc.tile_pool(name="ps_s2", bufs=2, space="PSUM") as ps_s2,
            tc.tile_pool(name="ps_ot", bufs=2, space="PSUM") as ps_ot,
        ):
            ident_f = consts.tile([128, 128], F32)
            make_identity(nc, ident_f)
            ident_b = consts.tile([128, 128], BF16)
            nc.vector.tensor_copy(ident_b, ident_f)
            # block-ones for the pair-merged reciprocal broadcast: head hp's
            # reciprocal row lives at partition 32*hp (partition APs must
            # start at a multiple of 32); sb2 routes row 32*hp to output
            # partitions [64hp, 64hp+64) and the zero rows contribute nothing.
            sb2_f = consts.tile([33, 128], F32)
            nc.vector.memset(sb2_f, 0.0)
            nc.vector.memset(sb2_f[0:1, 0:64], 1.0)
            nc.vector.memset(sb2_f[32:33, 64:128], 1.0)
            sb2 = consts.tile([33, 128], F32R)
            nc.vector.tensor_copy(sb2, sb2_f)

            # triangular 0/1 mask, duplicated for both heads of a pair:
            # pt2's diagonal 128-col sub-block is multiplied by it after exp
            # (1 where col >= row, else 0) — cheaper than adding -1e30 on the
            # PE, which put a full-row matmul into the score stream.
            mask_f = consts.tile([128, 2, 128], F32)
            nc.gpsimd.memset(mask_f, 1.0)
            nc.gpsimd.affine_select(
                out=mask_f, in_=mask_f,
                compare_op=mybir.AluOpType.is_ge, fill=0.0,
                base=0, pattern=[[0, 2], [1, 128]], channel_multiplier=-1)
            mask01 = consts.tile([128, 2, 128], BF16)
            nc.vector.tensor_copy(mask01, mask_f)

            w_all = consts.tile([128, CK, 3, HPC * D], BF16)
            wp_t = consts.tile([128, NPAIR, C], BF16)

            def emit_w_dmas():
                for ck in range(CK):
                    nc.sync.dma_start(out=w_all[:, ck, :, :], in_=wqkv_d[ck])
                nc.sync.dma_start(out=wp_t, in_=wp_d[:])

            # persistent per-core activations (bf16)
            qT = qkv.tile([128, NPAIR, T], BF16, tag="qT")
            kT = qkv.tile([128, NPAIR, T], BF16, tag="kT")
            vT = qkv.tile([128, NPAIR, T], BF16, tag="vT")
            vn = [vnp.tile([128, NJT, 65], BF16, tag=f"vn{h}", name=f"vn{h}")
                  for h in range(HPC)]
            for h in range(HPC):
                nc.vector.memset(vn[h][:, :, 64], 1.0)  # denominator ones col
            otn = qkv.tile([128, NPAIR, T], BF16, tag="otn")

            def make_proj(tb):
                """Emission units for token-block tb's q/k/v projections."""
                s = slice(tb * 512, (tb + 1) * 512)
                units = []
                tiles = {}

                def u_dma(tb=tb, s=s):
                    xT = xtp.tile([128, CK, 512], BF16, tag="xT", name="xT")
                    for g in range(2):
                        nc.sync.dma_start(
                            out=xT[:, 4 * g:4 * g + 4, :],
                            in_=xt_d[4 * g:4 * g + 4, :, s].rearrange(
                                "k c f -> c k f"))
                    tiles["xT"] = xT
                units.append(u_dma)

                for t, dst in ((0, qT), (1, kT), (2, vT)):
                    for oc in range(NPAIR):
                        def u_mm_a(t=t, oc=oc):
                            xT = tiles["xT"]
                            acc = ps_ot.tile([128, 512], F32, tag="aux",
                                             name="acc")
                            for ck in range(CK // 2):
                                nc.tensor.matmul(
                                    acc,
                                    w_all[:, ck, t, 128 * oc:128 * oc + 128],
                                    xT[:, ck, :],
                                    start=(ck == 0), stop=False)
                            tiles[("acc", t, oc)] = acc
                        def u_mm_b(t=t, oc=oc, dst=dst, s=s):
                            xT = tiles["xT"]
                            acc = tiles.pop(("acc", t, oc))
                            for ck in range(CK // 2, CK):
                                nc.tensor.matmul(
                                    acc,
                                    w_all[:, ck, t, 128 * oc:128 * oc + 128],
                                    xT[:, ck, :],
                                    start=False, stop=(ck == CK - 1))
                            nc.vector.tensor_copy(dst[:, oc, s], acc)
                        units.append(u_mm_a)
                        units.append(u_mm_b)

                for p in range(NPAIR):
                    def u_tr(p=p, tb=tb):
                        # interleave the two heads' transposes: disjoint PE
                        # row-groups (0-63 / 64-127), different PSUM banks,
                        # so adjacent pairs overlap in the array.
                        tr = [ps_ot.tile([128, 4, 64], BF16, tag="aux",
                                         name=f"tr{hp}") for hp in range(2)]
                        with nc.allow_low_precision(
                                reason="transpose, no accumulation"):
                            for k in range(4):
                                jt = tb * 4 + k
                                for hp in range(2):
                                    nc.tensor.transpose(
                                        tr[hp][:, k, :],
                                        vT[hp * 64:(hp + 1) * 64, p,
                                           jt * 128:(jt + 1) * 128],
                                        ident_b[hp * 64:(hp + 1) * 64,
                                                hp * 64:(hp + 1) * 64])
                        for hp in range(2):
                            nc.vector.tensor_copy(
                                vn[2 * p + hp][:, tb * 4:(tb + 1) * 4, 0:64],
                                tr[hp])
                    units.append(u_tr)
                return units

            def emit_yproj_units(ib):
                """Output projection for row-block ib (normalized otn)."""
                units = []
                for k in range(4):
                    it = ib * 4 + k

                    def u_y(it=it):
                        ys = ypool.tile([128, C], BF16, tag="y", name="ys")
                        for cb in range(2):
                            yp = ps_ot.tile([128, 512], F32, tag="aux",
                                            name="yp")
                            for p in range(NPAIR):
                                nc.tensor.matmul(
                                    yp,
                                    otn[:, p, it * 128:(it + 1) * 128],
                                    wp_t[:, p, cb * 512:(cb + 1) * 512],
                                    start=(p == 0), stop=(p == NPAIR - 1))
                            nc.vector.tensor_copy(
                                ys[:, cb * 512:(cb + 1) * 512], yp)
                        nc.sync.dma_start(
                            out=out_d[it * 128:(it + 1) * 128, :], in_=ys)
                    units.append(u_y)
                return units

            def attention(ib, interleave):
                """Attention for row-block ib; pops `interleave` units
                between jc iterations."""
                s0 = ib * 512
                njc = 4 * (ib + 1)
                n_steps = NPAIR * njc
                step = 0
                emitted = 0
                units = list(interleave)

                def pace():
                    nonlocal emitted
                    want = (step * len(units)) // max(n_steps - 2, 1)
                    while emitted < min(want, len(units)):
                        units[emitted]()
                        emitted += 1

                for p in range(NPAIR):
                    ot = [ps_ot.tile([65, 512], F32, tag="ot",
                                     name=f"ot{hp}") for hp in range(2)]

                    def mm1(jc, p=p):
                        m = jc - 4 * ib
                        diag = m >= 0
                        q0 = 128 * m if diag else 0
                        s2 = ps_s2.tile([128, 2, 512], F32, tag="s2",
                                        name="s2")
                        # the two heads' score matmuls sit in disjoint PE
                        # row-groups (0-63 / 64-127) and run concurrently
                        for hp in range(2):
                            nc.tensor.matmul(
                                s2[:, hp, q0:512],
                                kT[hp * 64:(hp + 1) * 64, p,
                                   jc * 128:(jc + 1) * 128],
                                qT[hp * 64:(hp + 1) * 64, p,
                                   s0 + q0:s0 + 512],
                                start=True, stop=True)
                        pt2 = ppool.tile([128, 2, 512], BF16, tag="p",
                                         name="pt2")
                        nc.scalar.activation(pt2[:, :, q0:512],
                                             s2[:, :, q0:512],
                                             ActF.Exp, scale=D ** -0.5)
                        if diag:
                            # zero the disallowed upper triangle of the
                            # 128-col diagonal sub-block (both heads at once)
                            nc.vector.tensor_mul(
                                pt2[:, :, q0:q0 + 128],
                                pt2[:, :, q0:q0 + 128], mask01)
                        return q0, pt2

                    def mm2(jc, q0, pt2, p=p, ot=ot):
                        for hp in range(2):
                            nc.tensor.matmul(
                                ot[hp][:, q0:512],
                                vn[2 * p + hp][:, jc, :],
                                pt2[:, hp, q0:512],
                                start=(jc == 0), stop=(jc == njc - 1),
                                skip_group_check=True)

                    pend = []
                    for jc in range(njc):
                        pend.append((jc, *mm1(jc)))
                        if len(pend) > 3:
                            j0, q0, p0 = pend.pop(0)
                            mm2(j0, q0, p0)
                        step += 1
                        pace()
                    for j0, q0, p0 in pend:
                        mm2(j0, q0, p0)

                    # normalization: divide by the denominator row (65th) —
                    # reciprocal, broadcast across 64 partitions via a tiny
                    # PE matmul, then multiply into otn (bf16). One matmul
                    # broadcasts both heads' reciprocals: block-ones sb2
                    # routes rec2 row hp to output partitions [64hp, 64hp+64).
                    otus = []
                    for hp in range(2):
                        otu = nrm.tile([65, 512], F32, tag="otu", name="otu")
                        nc.vector.tensor_copy(otu, ot[hp])
                        otus.append(otu)
                    rec2 = nrm.tile([33, 512], F32R, tag="rec", name="rec2")
                    with nc.allow_low_precision(
                            reason="f32r holds full fp32 bits"):
                        for hp in range(2):
                            nc.vector.reciprocal(
                                rec2[32 * hp:32 * hp + 1, :],
                                otus[hp][64:65, :])
                    rb = ps_ot.tile([128, 512], F32, tag="aux", name="rb")
                    nc.tensor.matmul(rb, sb2, rec2, start=True, stop=True)
                    for hp in range(2):
                        nc.vector.tensor_mul(
                            otn[hp * 64:(hp + 1) * 64, p, s0:s0 + 512],
                            otus[hp][0:64, :],
                            rb[hp * 64:(hp + 1) * 64, :])

                while emitted < len(units):
                    units[emitted]()
                    emitted += 1

            proj0 = make_proj(0)
            proj0[0]()          # xT(0) load first, then weights behind it
            emit_w_dmas()
            for u in proj0[1:]:
                u()
            # All yproj work is deferred into the last row-block's attention:
            # ib 0-2 are PE-bound (proj filler), while ib 3 has 64 exp-steps
            # with an otherwise idle PE — the deferred yproj fills it there.
            for ib in range(NIB):
                fillers = []
                if ib + 1 < NIB:
                    fillers += make_proj(ib + 1)
                else:
                    for j in range(NIB - 1):
                        fillers += emit_yproj_units(j)
                attention(ib, fillers)
            for u in emit_yproj_units(NIB - 1):
                u()

    split_excess_waits(nc)
    return nc


# ---------------------------------------------------------------------------
# Host-side: sharding, PJRT runner (compiled once per process), gather.
# ---------------------------------------------------------------------------

class _Runner:
    def __init__(self, B, T):
        import jax
        from jax.experimental.shard_map import shard_map
        from jax.sharding import Mesh, PartitionSpec
        from concourse.bass2jax import (_bass_exec_p, install_neuronx_cc_hook,
                                        partition_id_tensor)

        install_neuronx_cc_hook()
        nc = build_nc(B, T)
        self.nc = nc
        in_names, out_names, out_avals, zero_outs = [], [], [], []
        partition_name = (nc.partition_id_tensor.name
                          if nc.partition_id_tensor else None)
        for alloc in nc.m.functions[0].allocations:
            if not isinstance(alloc, mybir.MemoryLocationSet):
                continue
            name = alloc.memorylocations[0].name
            if alloc.kind == "ExternalInput":
                if name != partition_name:
                    in_names.append(name)
            elif alloc.kind == "ExternalOutput":
                out_names.append(name)
                shape = tuple(alloc.tensor_shape)
                dtype = mybir.dt.np(alloc.dtype)
                out_avals.append(jax.core.ShapedArray(shape, dtype))
                zero_outs.append(np.zeros(shape, dtype))
        self.in_names = list(in_names)
        self.out_names = out_names
        self.out_shapes = [tuple(a.shape) for a in out_avals]
        all_in_names = in_names + out_names
        if partition_name is not None:
            all_in_names.append(partition_name)

        def _body(*args):
            operands = list(args)
            if partition_name is not None:
                operands.append(partition_id_tensor())
            outs = _bass_exec_p.bind(
                *operands,
                out_avals=tuple(out_avals),
                in_names=tuple(all_in_names),
                out_names=tuple(out_names),
                lowering_input_output_aliases=(),
                sim_require_finite=True,
                sim_require_nnan=True,
                nc=nc,
            )
            return tuple(outs)

        devices = jax.devices()[:NCORES]
        self.mesh = Mesh(np.asarray(devices), ("core",))
        n_in = len(in_names) + len(out_names)
        self.fn = jax.jit(shard_map(
            _body, mesh=self.mesh,
            in_specs=(PartitionSpec("core"),) * n_in,
            out_specs=(PartitionSpec("core"),) * len(out_names),
            check_rep=False,
        ), keep_unused=True)
        self.zero_outs = zero_outs
        self._jax = jax

    def prepare(self, in_maps):
        """Concat per-core inputs along axis 0 and device_put."""
        jax = self._jax
        from jax.sharding import NamedSharding, PartitionSpec
        sh = NamedSharding(self.mesh, PartitionSpec("core"))
        args = []
        for i, name in enumerate(self.in_names):
            cat = np.concatenate([np.asarray(m[name]) for m in in_maps], axis=0)
            args.append(jax.device_put(cat, sh))
        for z in self.zero_outs:
            zz = np.zeros((NCORES * z.shape[0], *z.shape[1:]), z.dtype)
            args.append(jax.device_put(zz, sh))
        return args

    def run(self, args):
        outs = self.fn(*args)
        self._jax.block_until_ready(outs)
        return outs

    def split_outs(self, outs):
        res = []
        for c in range(NCORES):
            d = {}
            for i, name in enumerate(self.out_names):
                d[name] = np.asarray(outs[i]).reshape(
                    NCORES, *self.out_shapes[i])[c]
            res.append(d)
        return res


@functools.lru_cache(maxsize=2)
def _get_runner(B, T):
    return _Runner(B, T)


def make_in_maps(x, Wq, Wk, Wv, Wp):
    """Per-core input dicts from full tensors (host-side shard prep)."""
    import ml_dtypes
    bf16 = ml_dtypes.bfloat16
    x = np.asarray(x, np.float32)
    B, T, _ = x.shape
    Wq, Wk, Wv = (np.asarray(w, np.float32) for w in (Wq, Wk, Wv))
    Wp = np.asarray(Wp, np.float32)
    in_maps = []
    for c in range(NCORES):
        b = c // 2
        hs = slice((c % 2) * HPC, (c % 2) * HPC + HPC)
        xt = x[b].T.reshape(CK, 128, T)                    # [CK, 128, T]
        wqkv = np.stack([Wq[hs], Wk[hs], Wv[hs]])          # [3, HPC, C, D]
        wqkv = wqkv.reshape(3, HPC, CK, 128, D)
        wqkv = wqkv.transpose(2, 3, 0, 1, 4).reshape(CK, 128, 3, HPC * D)
        wp = Wp[(c % 2) * HPC * D:(c % 2) * HPC * D + HPC * D]  # [512, C]
        wp = wp.reshape(NPAIR, 128, C).transpose(1, 0, 2)  # [128, NPAIR, C]
        in_maps.append({
            "xt": np.ascontiguousarray(xt).astype(bf16),
            "wqkv": np.ascontiguousarray(wqkv).astype(bf16),
            "wp": np.ascontiguousarray(wp).astype(bf16),
        })
    return in_maps


def kernel(x, Wq, Wk, Wv, Wp, bp):
    B, T, _ = x.shape
    runner = _get_runner(B, T)
    args = runner.prepare(make_in_maps(x, Wq, Wk, Wv, Wp))
    outs = runner.run(args)
    per_core = runner.split_outs(outs)
    bp = np.asarray(bp, np.float32)
    res = np.empty((B, T, C), np.float32)
    for b in range(B):
        res[b] = (per_core[2 * b]["out"].astype(np.float32)
                  + per_core[2 * b + 1]["out"].astype(np.float32) + bp)
    return res


# revision 18
# speedup vs baseline: 2.2539x; 2.2539x over previous
"""Multi-head causal self-attention (B=4, T=2048, C=1024, H=16, D=64) on 8
Trainium2 NeuronCores.

Sharding: hybrid (batch x head-half). Core c handles batch c//2 and heads
[8*(c%2), 8*(c%2)+8). Each core computes q/k/v projections for its 8 heads,
causal attention, and its half of the output projection rows (512 of 1024).
The host sums each batch's two partial projections (bf16) and adds the bias.

Design highlights:
- bf16 activations/weights on chip (f32 PSUM accumulate): halves DMA + SBUF.
  Per-core DMA ~12MB (x slice 4MB bf16, partial out 4MB bf16, weights 4MB).
- Score matmuls for a head-pair sit at partitions 0-63 / 64-127, so the PE
  runs them concurrently in disjoint row-groups (auto tile_position).
- Diagonal blocks narrowed: only columns [128m:512] are computed/exp'd; the
  causal mask is a 0/1 bf16 triangle multiplied into the probabilities on
  the vector engine after exp (keeps full-row matmuls out of the PE stream).
- AV matmul carries a ones column in vn (65th output row = softmax
  denominator); normalization broadcasts both heads' reciprocals across
  their 64-partition halves with one block-ones PE matmul (partition APs
  must start at multiples of 32, hence the 33-row rec2/sb2 tiles).
- Emission interleaving: next token-block's projections fill the PE during
  attention; all output-projection work is deferred into the last row-block
  where the scalar engine (exp) is the local bottleneck.

A post-pass splits excess semaphore waits (walrus accepts one sync wait per
instruction for several instruction structs); excess waits move onto injected
NoOps on the same engine queue.
"""
import functools

import numpy as np

import concourse.bass as bass
import concourse.mybir as mybir
import concourse.tile as tile
from concourse.masks import make_identity

F32 = mybir.dt.float32
F32R = mybir.dt.float32r
BF16 = mybir.dt.bfloat16

C, H, D = 1024, 16, 64
NCORES = 8
HPC = 8                    # heads per core
NPAIR = HPC // 2           # head-pairs per core = 4
CK = C // 128              # contraction chunks = 8
ActF = mybir.ActivationFunctionType


def split_excess_waits(nc):
    """Move all-but-one sync wait of every instruction onto injected NoOps."""
    n_split = 0
    for f in nc.m.functions:
        for blk in f.blocks:
            out, changed = [], False
            for inst in blk.instructions:
                si = inst.sync_info
                if si is not None and len(si.on_wait) > 1:
                    for w_ in si.on_wait[:-1]:
                        nop = mybir.InstNoOp(name=f"I-wsplit-{n_split}")
                        n_split += 1
                        nop.engine = inst.engine
                        nop.sync_info = mybir.SyncInfo(on_wait=[w_], on_update=[])
                        out.append(nop)
                    inst.sync_info = mybir.SyncInfo(
                        on_wait=si.on_wait[-1:], on_update=si.on_update)
                    changed = True
                out.append(inst)
            if changed:
                blk.instructions = out
    return n_split


def build_nc(B, T):
    """One SPMD program; all 8 cores run it on different data slices."""
    NIB = T // 512           # query row-blocks = 4
    NJT = T // 128           # key blocks = 16
    nc = bass.Bass()

    xt_d = nc.dram_tensor("xt", [CK, 128, T], BF16, kind="ExternalInput")
    wqkv_d = nc.dram_tensor("wqkv", [CK, 128, 3, HPC * D], BF16,
                            kind="ExternalInput")
    wp_d = nc.dram_tensor("wp", [128, NPAIR, C], BF16, kind="ExternalInput")
    out_d = nc.dram_tensor("out", [T, C], BF16, kind="ExternalOutput")

    with tile.TileContext(nc) as tc:
        with (
            tc.tile_pool(name="consts", bufs=1) as consts,
            tc.tile_pool(name="xtp", bufs=3) as xtp,
            tc.tile_pool(name="qkv", bufs=1) as qkv,
            tc.tile_pool(name="vnp", bufs=1) as vnp,
            tc.tile_pool(name="pp", bufs=8) as ppool,
            tc.tile_pool(name="nrm", bufs=6) as nrm,
            tc.tile_pool(name="yp", bufs=3) as ypool,
            tc.tile_pool(name="ps_s2", bufs=2, space="PSUM") as ps_s2,
            tc.tile_pool(name="ps_ot", bufs=2, space="PSUM") as ps_ot,
        ):
            ident_f = consts.tile([128, 128], F32)
            make_identity(nc, ident_f)
            ident_b = consts.tile([128, 128], BF16)
            nc.vector.tensor_copy(ident_b, ident_f)
            # block-ones for the pair-merged reciprocal broadcast: head hp's
            # reciprocal row lives at partition 32*hp (partition APs must
            # start at a multiple of 32); sb2 routes row 32*hp to output
            # partitions [64hp, 64hp+64) and the zero rows contribute nothing.
            sb2_f = consts.tile([33, 128], F32)
            nc.vector.memset(sb2_f, 0.0)
            nc.vector.memset(sb2_f[0:1, 0:64], 1.0)
            nc.vector.memset(sb2_f[32:33, 64:128], 1.0)
            sb2 = consts.tile([33, 128], F32R)
            nc.vector.tensor_copy(sb2, sb2_f)

            # triangular 0/1 mask, duplicated for both heads of a pair:
            # pt2's diagonal 128-col sub-block is multiplied by it after exp
            # (1 where col >= row, else 0) — cheaper than adding -1e30 on the
            # PE, which put a full-row matmul into the score stream.
            mask_f = consts.tile([128, 2, 128], F32)
            nc.gpsimd.memset(mask_f, 1.0)
            nc.gpsimd.affine_select(
                out=mask_f, in_=mask_f,
                compare_op=mybir.AluOpType.is_ge, fill=0.0,
                base=0, pattern=[[0, 2], [1, 128]], channel_multiplier=-1)
            mask01 = consts.tile([128, 2, 128], BF16)
            nc.vector.tensor_copy(mask01, mask_f)

            w_all = consts.tile([128, CK, 3, HPC * D], BF16)
            wp_t = consts.tile([128, NPAIR, C], BF16)

            def emit_w_dmas():
                for ck in range(CK):
                    nc.sync.dma_start(out=w_all[:, ck, :, :], in_=wqkv_d[ck])
                nc.sync.dma_start(out=wp_t, in_=wp_d[:])

            # persistent per-core activations (bf16)
            qT = qkv.tile([128, NPAIR, T], BF16, tag="qT")
            kT = qkv.tile([128, NPAIR, T], BF16, tag="kT")
            vT = qkv.tile([128, NPAIR, T], BF16, tag="vT")
            vn = [vnp.tile([128, NJT, 65], BF16, tag=f"vn{h}", name=f"vn{h}")
                  for h in range(HPC)]
            for h in range(HPC):
                nc.vector.memset(vn[h][:, :, 64], 1.0)  # denominator ones col
            otn = qkv.tile([128, NPAIR, T], BF16, tag="otn")
            # persistent reciprocal-pair tile: rows 0/32 are rewritten per
            # pair; rows 1-31 must be ZERO (not garbage — 0*inf = NaN in the
            # broadcast matmul), so zero the whole tile once here.
            rec2 = qkv.tile([33, 512], F32R, tag="rec2")
            rec2_z = consts.tile([33, 512], F32)
            nc.vector.memset(rec2_z, 0.0)
            with nc.allow_low_precision(reason="f32r holds full fp32 bits"):
                nc.vector.tensor_copy(rec2, rec2_z)

            def make_proj(tb):
                """Emission units for token-block tb's q/k/v projections."""
                s = slice(tb * 512, (tb + 1) * 512)
                units = []
                tiles = {}

                def u_dma(tb=tb, s=s):
                    xT = xtp.tile([128, CK, 512], BF16, tag="xT", name="xT")
                    for g in range(2):
                        nc.sync.dma_start(
                            out=xT[:, 4 * g:4 * g + 4, :],
                            in_=xt_d[4 * g:4 * g + 4, :, s].rearrange(
                                "k c f -> c k f"))
                    tiles["xT"] = xT
                units.append(u_dma)

                for t, dst in ((0, qT), (1, kT), (2, vT)):
                    for oc in range(NPAIR):
                        def u_mm_a(t=t, oc=oc):
                            xT = tiles["xT"]
                            acc = ps_ot.tile([128, 512], F32, tag="aux",
                                             name="acc")
                            for ck in range(CK // 2):
                                nc.tensor.matmul(
                                    acc,
                                    w_all[:, ck, t, 128 * oc:128 * oc + 128],
                                    xT[:, ck, :],
                                    start=(ck == 0), stop=False)
                            tiles[("acc", t, oc)] = acc
                        def u_mm_b(t=t, oc=oc, dst=dst, s=s):
                            xT = tiles["xT"]
                            acc = tiles.pop(("acc", t, oc))
                            for ck in range(CK // 2, CK):
                                nc.tensor.matmul(
                                    acc,
                                    w_all[:, ck, t, 128 * oc:128 * oc + 128],
                                    xT[:, ck, :],
                                    start=False, stop=(ck == CK - 1))
                            nc.vector.tensor_copy(dst[:, oc, s], acc)
                        units.append(u_mm_a)
                        units.append(u_mm_b)

                for p in range(NPAIR):
                    def u_tr(p=p, tb=tb):
                        # interleave the two heads' transposes: disjoint PE
                        # row-groups (0-63 / 64-127), different PSUM banks,
                        # so adjacent pairs overlap in the array.
                        tr = [ps_ot.tile([128, 4, 64], BF16, tag="aux",
                                         name=f"tr{hp}") for hp in range(2)]
                        with nc.allow_low_precision(
                                reason="transpose, no accumulation"):
                            for k in range(4):
                                jt = tb * 4 + k
                                for hp in range(2):
                                    nc.tensor.transpose(
                                        tr[hp][:, k, :],
                                        vT[hp * 64:(hp + 1) * 64, p,
                                           jt * 128:(jt + 1) * 128],
                                        ident_b[hp * 64:(hp + 1) * 64,
                                                hp * 64:(hp + 1) * 64])
                        for hp in range(2):
                            nc.vector.tensor_copy(
                                vn[2 * p + hp][:, tb * 4:(tb + 1) * 4, 0:64],
                                tr[hp])
                    units.append(u_tr)
                return units

            def emit_yproj_units(ib):
                """Output projection for row-block ib (normalized otn)."""
                units = []
                for k in range(4):
                    it = ib * 4 + k

                    def u_y(it=it):
                        ys = ypool.tile([128, C], BF16, tag="y", name="ys")
                        for cb in range(2):
                            yp = ps_ot.tile([128, 512], F32, tag="aux",
                                            name="yp")
                            for p in range(NPAIR):
                                nc.tensor.matmul(
                                    yp,
                                    otn[:, p, it * 128:(it + 1) * 128],
                                    wp_t[:, p, cb * 512:(cb + 1) * 512],
                                    start=(p == 0), stop=(p == NPAIR - 1))
                            nc.vector.tensor_copy(
                                ys[:, cb * 512:(cb + 1) * 512], yp)
                        nc.sync.dma_start(
                            out=out_d[it * 128:(it + 1) * 128, :], in_=ys)
                    units.append(u_y)
                return units

            def attention(ib, interleave):
                """Attention for row-block ib; pops `interleave` units
                between jc iterations."""
                s0 = ib * 512
                njc = 4 * (ib + 1)
                n_steps = NPAIR * njc
                step = 0
                emitted = 0
                units = list(interleave)

                def pace():
                    nonlocal emitted
                    want = (step * len(units)) // max(n_steps - 2, 1)
                    while emitted < min(want, len(units)):
                        units[emitted]()
                        emitted += 1

                for p in range(NPAIR):
                    ot = [ps_ot.tile([65, 512], F32, tag="ot",
                                     name=f"ot{hp}") for hp in range(2)]

                    def mm1(jc, p=p):
                        m = jc - 4 * ib
                        diag = m >= 0
                        q0 = 128 * m if diag else 0
                        s2 = ps_s2.tile([128, 2, 512], F32, tag="s2",
                                        name="s2")
                        # the two heads' score matmuls sit in disjoint PE
                        # row-groups (0-63 / 64-127) and run concurrently
                        for hp in range(2):
                            nc.tensor.matmul(
                                s2[:, hp, q0:512],
                                kT[hp * 64:(hp + 1) * 64, p,
                                   jc * 128:(jc + 1) * 128],
                                qT[hp * 64:(hp + 1) * 64, p,
                                   s0 + q0:s0 + 512],
                                start=True, stop=True)
                        pt2 = ppool.tile([128, 2, 512], BF16, tag="p",
                                         name="pt2")
                        nc.scalar.activation(pt2[:, :, q0:512],
                                             s2[:, :, q0:512],
                                             ActF.Exp, scale=D ** -0.5)
                        if diag:
                            # zero the disallowed upper triangle of the
                            # 128-col diagonal sub-block (both heads at once)
                            nc.vector.tensor_mul(
                                pt2[:, :, q0:q0 + 128],
                                pt2[:, :, q0:q0 + 128], mask01)
                        return q0, pt2

                    def mm2(jc, q0, pt2, p=p, ot=ot):
                        for hp in range(2):
                            nc.tensor.matmul(
                                ot[hp][:, q0:512],
                                vn[2 * p + hp][:, jc, :],
                                pt2[:, hp, q0:512],
                                start=(jc == 0), stop=(jc == njc - 1),
                                skip_group_check=True)

                    pend = []
                    for jc in range(njc):
                        pend.append((jc, *mm1(jc)))
                        if len(pend) > 3:
                            j0, q0, p0 = pend.pop(0)
                            mm2(j0, q0, p0)
                        step += 1
                        pace()
                    for j0, q0, p0 in pend:
                        mm2(j0, q0, p0)

                    # normalization: divide by the denominator row (65th) —
                    # reciprocal, broadcast across 64 partitions via a tiny
                    # PE matmul, then multiply into otn (bf16). One matmul
                    # broadcasts both heads' reciprocals: block-ones sb2
                    # routes rec2 row hp to output partitions [64hp, 64hp+64).
                    otus = []
                    for hp in range(2):
                        otu = nrm.tile([65, 512], F32, tag="otu", name="otu")
                        nc.vector.tensor_copy(otu, ot[hp])
                        otus.append(otu)
                    with nc.allow_low_precision(
                            reason="f32r holds full fp32 bits"):
                        for hp in range(2):
                            nc.vector.reciprocal(
                                rec2[32 * hp:32 * hp + 1, :],
                                otus[hp][64:65, :])
                    rb = ps_ot.tile([128, 512], F32, tag="aux", name="rb")
                    nc.tensor.matmul(rb, sb2, rec2, start=True, stop=True)
                    for hp in range(2):
                        nc.vector.tensor_mul(
                            otn[hp * 64:(hp + 1) * 64, p, s0:s0 + 512],
                            otus[hp][0:64, :],
                            rb[hp * 64:(hp + 1) * 64, :])

                while emitted < len(units):
                    units[emitted]()
                    emitted += 1

            proj0 = make_proj(0)
            proj0[0]()          # xT(0) load first, then weights behind it
            emit_w_dmas()
            for u in proj0[1:]:
                u()
            # All yproj work is deferred into the last row-block's attention:
            # ib 0-2 are PE-bound (proj filler), while ib 3 has 64 exp-steps
            # with an otherwise idle PE — the deferred yproj fills it there.
            for ib in range(NIB):
                fillers = []
                if ib + 1 < NIB:
                    fillers += make_proj(ib + 1)
                else:
                    for j in range(NIB - 1):
                        fillers += emit_yproj_units(j)
                attention(ib, fillers)
            for u in emit_yproj_units(NIB - 1):
                u()

    split_excess_waits(nc)
    return nc


# ---------------------------------------------------------------------------
# Host-side: sharding, PJRT runner (compiled once per process), gather.
# ---------------------------------------------------------------------------

class _Runner:
    def __init__(self, B, T):
        import jax
        from jax.experimental.shard_map import shard_map
        from jax.sharding import Mesh, PartitionSpec
        from concourse.bass2jax import (_bass_exec_p, install_neuronx_cc_hook,
                                        partition_id_tensor)

        install_neuronx_cc_hook()
        nc = build_nc(B, T)
        self.nc = nc
        in_names, out_names, out_avals, zero_outs = [], [], [], []
        partition_name = (nc.partition_id_tensor.name
                          if nc.partition_id_tensor else None)
        for alloc in nc.m.functions[0].allocations:
            if not isinstance(alloc, mybir.MemoryLocationSet):
                continue
            name = alloc.memorylocations[0].name
            if alloc.kind == "ExternalInput":
                if name != partition_name:
                    in_names.append(name)
            elif alloc.kind == "ExternalOutput":
                out_names.append(name)
                shape = tuple(alloc.tensor_shape)
                dtype = mybir.dt.np(alloc.dtype)
                out_avals.append(jax.core.ShapedArray(shape, dtype))
                zero_outs.append(np.zeros(shape, dtype))
        self.in_names = list(in_names)
        self.out_names = out_names
        self.out_shapes = [tuple(a.shape) for a in out_avals]
        all_in_names = in_names + out_names
        if partition_name is not None:
            all_in_names.append(partition_name)

        def _body(*args):
            operands = list(args)
            if partition_name is not None:
                operands.append(partition_id_tensor())
            outs = _bass_exec_p.bind(
                *operands,
                out_avals=tuple(out_avals),
                in_names=tuple(all_in_names),
                out_names=tuple(out_names),
                lowering_input_output_aliases=(),
                sim_require_finite=True,
                sim_require_nnan=True,
                nc=nc,
            )
            return tuple(outs)

        devices = jax.devices()[:NCORES]
        self.mesh = Mesh(np.asarray(devices), ("core",))
        n_in = len(in_names) + len(out_names)
        self.fn = jax.jit(shard_map(
            _body, mesh=self.mesh,
            in_specs=(PartitionSpec("core"),) * n_in,
            out_specs=(PartitionSpec("core"),) * len(out_names),
            check_rep=False,
        ), keep_unused=True)
        self.zero_outs = zero_outs
        self._jax = jax

    def prepare(self, in_maps):
        """Concat per-core inputs along axis 0 and device_put."""
        jax = self._jax
        from jax.sharding import NamedSharding, PartitionSpec
        sh = NamedSharding(self.mesh, PartitionSpec("core"))
        args = []
        for i, name in enumerate(self.in_names):
            cat = np.concatenate([np.asarray(m[name]) for m in in_maps], axis=0)
            args.append(jax.device_put(cat, sh))
        for z in self.zero_outs:
            zz = np.zeros((NCORES * z.shape[0], *z.shape[1:]), z.dtype)
            args.append(jax.device_put(zz, sh))
        return args

    def run(self, args):
        outs = self.fn(*args)
        self._jax.block_until_ready(outs)
        return outs

    def split_outs(self, outs):
        res = []
        for c in range(NCORES):
            d = {}
            for i, name in enumerate(self.out_names):
                d[name] = np.asarray(outs[i]).reshape(
                    NCORES, *self.out_shapes[i])[c]
            res.append(d)
        return res


@functools.lru_cache(maxsize=2)
def _get_runner(B, T):
    return _Runner(B, T)


def make_in_maps(x, Wq, Wk, Wv, Wp):
    """Per-core input dicts from full tensors (host-side shard prep)."""
    import ml_dtypes
    bf16 = ml_dtypes.bfloat16
    x = np.asarray(x, np.float32)
    B, T, _ = x.shape
    Wq, Wk, Wv = (np.asarray(w, np.float32) for w in (Wq, Wk, Wv))
    Wp = np.asarray(Wp, np.float32)
    in_maps = []
    for c in range(NCORES):
        b = c // 2
        hs = slice((c % 2) * HPC, (c % 2) * HPC + HPC)
        xt = x[b].T.reshape(CK, 128, T)                    # [CK, 128, T]
        wqkv = np.stack([Wq[hs], Wk[hs], Wv[hs]])          # [3, HPC, C, D]
        wqkv = wqkv.reshape(3, HPC, CK, 128, D)
        wqkv = wqkv.transpose(2, 3, 0, 1, 4).reshape(CK, 128, 3, HPC * D)
        wp = Wp[(c % 2) * HPC * D:(c % 2) * HPC * D + HPC * D]  # [512, C]
        wp = wp.reshape(NPAIR, 128, C).transpose(1, 0, 2)  # [128, NPAIR, C]
        in_maps.append({
            "xt": np.ascontiguousarray(xt).astype(bf16),
            "wqkv": np.ascontiguousarray(wqkv).astype(bf16),
            "wp": np.ascontiguousarray(wp).astype(bf16),
        })
    return in_maps


def kernel(x, Wq, Wk, Wv, Wp, bp):
    B, T, _ = x.shape
    runner = _get_runner(B, T)
    args = runner.prepare(make_in_maps(x, Wq, Wk, Wv, Wp))
    outs = runner.run(args)
    per_core = runner.split_outs(outs)
    bp = np.asarray(bp, np.float32)
    res = np.empty((B, T, C), np.float32)
    for b in range(B):
        res[b] = (per_core[2 * b]["out"].astype(np.float32)
                  + per_core[2 * b + 1]["out"].astype(np.float32) + bp)
    return res
